# revision 68
# baseline (speedup 1.0000x reference)
"""Trainium2 Bass kernel for MultiHeadSelfAttentionModelV1.

Model (per batch row):
    e   = emb_table[x]                      # [S, E]
    Q/K/V = e @ W* + b*                     # [S, E], split into H heads of Dh
    P_h = softmax(Q_h K_h^T / sqrt(Dh))
    ctx = concat_h(P_h V_h) @ Wo + bo       # [S, E]
    out = max_tokens(ctx) @ Wc + bc         # [OUT]

Sharding: pure data parallel over batch; B == n_cores == 8, one row per core.

Measured: ~370 us HW (vs 420 us baseline), rel err 8.1e-3 (gate 2e-2).
(A ~313 us variant using a stride-0 broadcast DMA from a DRAM bounce for
the softmax-denominator fanout was rejected: it contained the kernel's
only DMA-write -> DMA-read dependency edge and produced rare (~1/14
runs) corrupted outputs; all edges below are engine-mediated.)

Key design choices (numerics validated in numpy sim, sim == HW err):
 - emb table bf16 in DRAM; token gather via indirect DMA, PE-transpose in
   bf16, ONE strided eviction per token tile into a single eT supertile.
 - Q/K/V projections bf16 (fp8 DoubleRow was tried: only -4 us, err
   1.6e-2 — rejected). Bias folds into the PSUM eviction (ACT
   activation-Identity-bias for Q, DVE tensor_scalar_add for K).
 - V is evicted to fp8(e4m3) pair tiles va2[p][128 tok, 8 heads x 2
   k-subtiles x 80] laid out [V_h (64) | 1 | pad15] (DoubleRow ldweights
   needs subtile step % 16 == 0); the ones column makes the PV matmul
   accumulate the softmax denominator in PSUM row 64 for free.
 - softmax exp is ONE fused op per [128,1024] score tile:
   u8 = s*(1/ln2) + 55.35, bitcast u8 -> e4m3 IS exp(s/8) (Schraudolph in
   fp8 bit space). The quantization bias cancels in the softmax
   denominator because the denominator sums the same quantized P. Runs on
   ACT (activation Copy w/ scale+bias -> u8) or DVE (tensor_scalar
   mult,add -> u8), ONE engine per (j, head-pair) unit so every softmax
   row sees a single convert-rounding mode. GPSIMD cannot read PSUM on
   TRN2, so only these two engines can exp; ~145 us/engine is the floor.
 - PV runs fp8 DoubleRow: two 128-token k-tiles per matmul = 2x fewer PE
   streaming cycles. ctx accumulates [65, 512] fp32 in one PSUM bank.
 - Two-pass heads per unit: the chunk loop accumulates only head-e; the
   SBUF pt tiles are replayed through 8 more DoubleRow matmuls for head-o.
   This halves live ctx banks (2 instead of 4), buying a THIRD stile slot
   (sps bufs=3 x 2 banks + ctx 2 banks = all 8 PSUM banks) so scores(k+1)
   overlaps exp(k) instead of serializing on the slot.
 - Phase C is software-pipelined: two (j, head-pair) units in flight,
   ACT/DVE alternating, interleaved at chunk granularity. Q/K projection
   blocks are emitted lazily as unit prerequisites (their PE work fills
   exp-bound slack).
 - Normalization: ctx evicted on the unit's OWN exp engine (no priority
   inversion), both denominator rows land contiguous in ctx_sb row 64 ->
   one DMA to [128,8] for partition-parallel DVE reciprocal (bf16 out) ->
   one DMA back to a [1,1024] row -> PE outer-product ones^T @ row
   broadcasts it into a borrowed PSUM stile slot (the gpsimd
   partition_broadcast costs 5.5 us of ucode time and stalled the PE at
   every j boundary) -> own-engine copy to SBUF -> two GPSIMD multiplies
   write normalized bf16 ctx^T into CT. The outer-product + copy are
   emitted as a deferred pending-op (~7 scheduler steps) so the in-order
   PE stream never waits on the reciprocal row.
 - Output projection + maxpool run per j-chunk, emission deferred ~20
   scheduler steps so the normalize chains complete in the shadow of the
   next j's PE work. bo is folded into the classifier bias on the host
   (max commutes with the per-feature constant bo).
"""

import sys

import numpy as np

if "/opt/trn_rl_repo" not in sys.path:
    sys.path.insert(0, "/opt/trn_rl_repo")

from collections import deque

import concourse.bass as bass
import concourse.bacc as bacc
import concourse.tile as tile
from concourse import mybir
from concourse.masks import make_identity

F32 = mybir.dt.float32
BF16 = mybir.dt.bfloat16
F8 = mybir.dt.float8e4
U8 = mybir.dt.uint8
I32 = mybir.dt.int32
ADD = mybir.AluOpType.add
MULT = mybir.AluOpType.mult
MAXOP = mybir.AluOpType.max
IDENT_FN = mybir.ActivationFunctionType.Identity
COPY_FN = mybir.ActivationFunctionType.Copy
DR = mybir.MatmulPerfMode.DoubleRow
X_AXIS = mybir.AxisListType.X

# exp(s/8) ~= bitcast_e4m3(u8(s * SCH_A + SCH_B)); the e4m3 bit pattern of
# exp(s/8) is affine in s (Schraudolph). Tuned in sim; robust to the
# (unknown) HW round-vs-truncate convert mode since the resulting global
# scale on P cancels in the softmax denominator.
SCH_A = 1.4426950408889634
SCH_B = 55.35

B = 8
E = 512
H = 8
DH = 64
OUT = 10
N_CORES = 8

# Exp engine per (j, head-pair) unit. GPSIMD cannot read PSUM on TRN2, so
# only ACT and DVE can run the exp-convert. One engine per unit keeps each
# softmax row on a single convert-rounding mode; the window-2 scheduler
# keeps both engines busy on alternating units.
ENG_UNITS = ["act", "dve"]


def build(S=2048, VOCAB=50257):
    """Build the per-core Bass program (same program on all 8 cores)."""
    nc = bacc.Bacc()

    NT = S // 128   # 128-token tiles (16)
    NJ = S // 512   # 512-token q-chunks (4)
    NE = E // 128   # 128-feature chunks (4)
    NP = NT // 2    # pairs of token tiles for DoubleRow (8)

    xi = nc.declare_dram_parameter("xi", [128, NT], I32, isOutput=False)
    emb = nc.declare_dram_parameter("emb", [VOCAB, E], BF16, isOutput=False)
    wq = nc.declare_dram_parameter("wq", [E, E], BF16, isOutput=False)
    wk = nc.declare_dram_parameter("wk", [E, E], BF16, isOutput=False)
    wv = nc.declare_dram_parameter("wv", [E, E], BF16, isOutput=False)
    wo = nc.declare_dram_parameter("wo", [E, E], BF16, isOutput=False)
    wc = nc.declare_dram_parameter("wc", [E, OUT], F32, isOutput=False)
    bq = nc.declare_dram_parameter("bq", [128, NE], F32, isOutput=False)
    bk = nc.declare_dram_parameter("bk", [128, NE], F32, isOutput=False)
    bo = nc.declare_dram_parameter("bo", [128, NE], F32, isOutput=False)
    bv = nc.declare_dram_parameter("bv", [1, E], BF16, isOutput=False)
    bc = nc.declare_dram_parameter("bc", [OUT, 1], F32, isOutput=False)
    out = nc.declare_dram_parameter("out", [OUT, 1], F32, isOutput=True)

    with tile.TileContext(nc) as tc:
        with (
            tc.tile_pool(name="consts", bufs=1) as consts,
            tc.tile_pool(name="qkT", bufs=1) as qkT_pool,
            tc.tile_pool(name="va2p", bufs=1) as va2_pool,
            tc.tile_pool(name="ctxT", bufs=1) as ctxT_pool,
            tc.tile_pool(name="eTp", bufs=1) as eT_pool,
            tc.tile_pool(name="projw", bufs=1) as projw,
            tc.tile_pool(name="fin", bufs=1) as fin_pool,
        ):
            # ---- constants (emission order = DMA priority: index + QKV
            # weights first so the gather/projection pipeline starts ASAP)
            idx_sb = consts.tile([128, NT], I32, tag="idx")
            nc.sync.dma_start(out=idx_sb, in_=xi[:, :])
            wq_sb = [projw.tile([128, E], BF16, tag=f"wq{k}", name=f"wq{k}")
                     for k in range(NE)]
            wk_sb = [projw.tile([128, E], BF16, tag=f"wk{k}", name=f"wk{k}")
                     for k in range(NE)]
            wv_sb = [projw.tile([128, E], BF16, tag=f"wv{k}", name=f"wv{k}")
                     for k in range(NE)]
            for k in range(NE):
                nc.sync.dma_start(out=wv_sb[k], in_=wv[k * 128:(k + 1) * 128, :])
                nc.sync.dma_start(out=wk_sb[k], in_=wk[k * 128:(k + 1) * 128, :])
                nc.sync.dma_start(out=wq_sb[k], in_=wq[k * 128:(k + 1) * 128, :])
            ident = consts.tile([128, 128], BF16, tag="ident")
            make_identity(nc, ident)
            wo_sb = [consts.tile([128, E], BF16, tag=f"wo{k}", name=f"wo{k}")
                     for k in range(NE)]
            for k in range(NE):
                nc.sync.dma_start(out=wo_sb[k], in_=wo[k * 128:(k + 1) * 128, :])
            wc_sb = [consts.tile([128, OUT], F32, tag=f"wc{k}", name=f"wc{k}")
                     for k in range(NE)]
            for k in range(NE):
                nc.sync.dma_start(out=wc_sb[k], in_=wc[k * 128:(k + 1) * 128, :])
            bq_sb = consts.tile([128, NE], F32, tag="bq")
            nc.sync.dma_start(out=bq_sb, in_=bq[:, :])
            bk_sb = consts.tile([128, NE], F32, tag="bk")
            nc.sync.dma_start(out=bk_sb, in_=bk[:, :])
            bo_sb = consts.tile([128, NE], F32, tag="bo")
            nc.sync.dma_start(out=bo_sb, in_=bo[:, :])
            bv_sb = consts.tile([1, E], BF16, tag="bv")
            nc.sync.dma_start(out=bv_sb, in_=bv[:, :])
            bc_sb = consts.tile([OUT, 1], F32, tag="bc")
            nc.sync.dma_start(out=bc_sb, in_=bc[:, :])
            ones_row = consts.tile([1, 128], BF16, tag="ones")
            nc.vector.memset(ones_row, 1.0)
            # e^T as ONE tile [128, NE*S] bf16 (feature chunk kk at columns
            # kk*S..): lets each token tile evict with a single strided copy.
            eT = eT_pool.tile([128, NE * S], BF16, tag="eT", name="eT")

            # persistent activations
            QT = [qkT_pool.tile([128, S], BF16, tag=f"qt{k}", name=f"qt{k}")
                  for k in range(NE)]
            KT = [qkT_pool.tile([128, S], BF16, tag=f"kt{k}", name=f"kt{k}")
                  for k in range(NE)]
            # V fp8 pair tiles: [128 tok, H * (2 k-subtiles * 80)]; per head
            # two [V_h | 1 | pad] blocks at stride 80 (DoubleRow ldweights
            # requires subtile step % 16 == 0). Preset to 1.0 so the ones
            # columns stay; pad columns are never read.
            va2 = [va2_pool.tile([128, H * 160], F8, tag=f"va{p}",
                                 name=f"va{p}") for p in range(NP)]
            CT = [ctxT_pool.tile([128, S], BF16, tag=f"ct{k}", name=f"ct{k}")
                  for k in range(NE)]

            # ============ phase A: gather, eT, V projection ============
            with (
                tc.tile_pool(name="enat", bufs=3) as enat_pool,
                tc.tile_pool(name="tps", bufs=2, space="PSUM") as tps,
                tc.tile_pool(name="qkvps", bufs=4, space="PSUM") as qkvps,
            ):
                for p in range(NP):
                    nc.vector.memset(va2[p][:], 1.0)

                evict_rr = ["dve", "act"]
                for t in range(NT):
                    e_nat = enat_pool.tile([128, E], BF16)
                    nc.gpsimd.indirect_dma_start(
                        out=e_nat[:],
                        out_offset=None,
                        in_=emb[:, :],
                        in_offset=bass.IndirectOffsetOnAxis(
                            ap=idx_sb[:, t:t + 1], axis=0
                        ),
                    )
                    # 4 transposes collect in one PSUM tile -> ONE eviction
                    tp = tps.tile([128, 512], BF16)
                    for f in range(NE):
                        nc.tensor.transpose(
                            out=tp[:, f * 128:(f + 1) * 128],
                            in_=e_nat[:, f * 128:(f + 1) * 128],
                            identity=ident[:],
                        )
                    dst = eT[:].rearrange(
                        "p (f s) -> p f s", s=S)[:, :, t * 128:(t + 1) * 128]
                    src = tp[:].rearrange("p (f c) -> p f c", c=128)
                    if evict_rr[t % 2] == "dve":
                        nc.vector.tensor_copy(out=dst, in_=src)
                    else:
                        nc.scalar.copy(out=dst, in_=src)

                # V token-major -> fp8 pair tiles (+bv via ones-row matmul)
                for t in range(NT):
                    ps = qkvps.tile([128, 512], F32, tag="qkv")
                    for kk in range(NE):
                        nc.tensor.matmul(
                            out=ps[:],
                            lhsT=eT[:, kk * S + t * 128:kk * S + (t + 1) * 128],
                            rhs=wv_sb[kk][:],
                            start=(kk == 0),
                            stop=False,
                        )
                    nc.tensor.matmul(
                        out=ps[:], lhsT=ones_row[:], rhs=bv_sb[:],
                        start=False, stop=True,
                    )
                    p, half = divmod(t, 2)
                    dst = va2[p][:].rearrange(
                        "p (h two c) -> p h two c", two=2, c=80)
                    nc.scalar.copy(
                        out=dst[:, :, half, 0:DH],
                        in_=ps[:].rearrange("p (h c) -> p h c", c=DH),
                    )

            # ============ phase C+D: attention, out-proj, pool ============
            with (
                tc.tile_pool(name="ptp", bufs=18) as pt_pool,
                tc.tile_pool(name="rep", bufs=2) as rep_pool,
                tc.tile_pool(name="sps", bufs=3, space="PSUM") as sps,
                tc.tile_pool(name="ctxps", bufs=2, space="PSUM") as ctxps,
            ):
                pooled = [fin_pool.tile([128, 1], F32, tag=f"pool{m}",
                                        name=f"pool{m}") for m in range(NE)]

                def emit_exp(eng, dst_u8, src):
                    if eng == "act":
                        nc.scalar.activation(
                            out=dst_u8, in_=src, func=COPY_FN,
                            scale=SCH_A, bias=SCH_B,
                        )
                    elif eng == "dve":
                        nc.vector.tensor_scalar(
                            out=dst_u8, in0=src,
                            scalar1=SCH_A, scalar2=SCH_B, op0=MULT, op1=ADD,
                        )
                    else:
                        nc.gpsimd.tensor_scalar(
                            out=dst_u8, in0=src,
                            scalar1=SCH_A, scalar2=SCH_B, op0=MULT, op1=ADD,
                        )

                # Q/K projection blocks are emitted lazily inside phase C,
                # right before the first unit that needs them — their PE
                # work fills exp-bound pipeline slack. They borrow sps slots.
                emitted_k = set()
                emitted_q = set()

                def emit_k_block(m, jj):
                    ps = sps.tile([128, 1024], F32, tag="s", name="kps")
                    for kk in range(NE):
                        nc.tensor.matmul(
                            out=ps[:, 0:512],
                            lhsT=wk_sb[kk][:, m * 128:(m + 1) * 128],
                            rhs=eT[:, kk * S + jj * 512:kk * S + (jj + 1) * 512],
                            start=(kk == 0),
                            stop=(kk == NE - 1),
                        )
                    nc.vector.tensor_scalar_add(
                        out=KT[m][:, jj * 512:(jj + 1) * 512],
                        in0=ps[:, 0:512], scalar1=bk_sb[:, m:m + 1],
                    )

                def emit_q_block(m, jj):
                    ps = sps.tile([128, 1024], F32, tag="s", name="qps")
                    for kk in range(NE):
                        nc.tensor.matmul(
                            out=ps[:, 0:512],
                            lhsT=wq_sb[kk][:, m * 128:(m + 1) * 128],
                            rhs=eT[:, kk * S + jj * 512:kk * S + (jj + 1) * 512],
                            start=(kk == 0),
                            stop=(kk == NE - 1),
                        )
                    nc.scalar.activation(
                        out=QT[m][:, jj * 512:(jj + 1) * 512],
                        in_=ps[:, 0:512], func=IDENT_FN,
                        bias=bq_sb[:, m:m + 1], scale=1.0,
                    )

                def ensure_proj(j, hp):
                    if hp not in emitted_k:
                        emitted_k.add(hp)
                        for jj in range(NJ):
                            emit_k_block(hp, jj)
                            yield
                    if (j, hp) not in emitted_q:
                        emitted_q.add((j, hp))
                        emit_q_block(hp, j)
                        yield

                def chain2_gen(delay, eng, hp, j, ctx_sb, rrow_b):
                    # Deferred second half of the normalize chain: by the
                    # time the PE reaches the outer-product, the reciprocal
                    # row is ready (no in-order PE stall). Every edge here
                    # is engine-mediated — no DMA-write -> DMA-read pairs.
                    for _ in range(delay):
                        yield
                    # broadcast = PE outer product ones[1,64]^T @ rrow[1,.]
                    rep_ps = sps.tile([128, 1024], F32, tag="s", name="repps")
                    for half in range(2):
                        nc.tensor.matmul(
                            out=rep_ps[0:64, half * 512:(half + 1) * 512],
                            lhsT=ones_row[0:1, 0:64],
                            rhs=rrow_b[:, half * 512:(half + 1) * 512],
                            start=True, stop=True,
                        )
                    rep_sb = rep_pool.tile([64, 1024], F32, tag="rep",
                                           bufs=4)
                    if eng == "act":
                        nc.scalar.copy(out=rep_sb[:], in_=rep_ps[0:64, :])
                    else:
                        nc.vector.tensor_copy(out=rep_sb[:],
                                              in_=rep_ps[0:64, :])
                    for off, h in ((0, 2 * hp), (512, 2 * hp + 1)):
                        nc.gpsimd.tensor_tensor(
                            out=CT[hp][(h % 2) * 64:(h % 2) * 64 + 64,
                                       j * 512:(j + 1) * 512],
                            in0=ctx_sb[0:DH, off:off + 512],
                            in1=rep_sb[:, off:off + 512],
                            op=MULT,
                        )

                def pv_matmul(ctx, p, h, ptf8, start, stop):
                    nc.tensor.matmul(
                        out=ctx[:],
                        lhsT=va2[p][:, h * 160:(h + 1) * 160]
                        .rearrange("p (two c) -> p two c", c=80)[:, :, 0:65],
                        rhs=ptf8,
                        start=start, stop=stop,
                        perf_mode=DR,
                        skip_group_check=True,
                    )

                def unit(j, hp, eng):
                    """One (j, head-pair) attention unit; yields per chunk.

                    Two-pass over heads: the chunk loop accumulates only
                    head-e (1 PSUM bank); the stored SBUF pt tiles are then
                    replayed through 8 more DR matmuls for head-o. This
                    halves live ctx banks, buying a third stile slot so
                    scores(k+1) overlaps exp(k).
                    """
                    yield from ensure_proj(j, hp)
                    ctx_e = ctxps.tile([DH + 1, 512], F32, tag="ctx",
                                       name="ctx_e")
                    pts = []
                    for p in range(NP):
                        # pt layout [128, (k-subtile, head, q)] = the raw
                        # concatenation of the pair's two exp outputs; each
                        # head's two k-subtile P blocks sit at stride 1024,
                        # which DoubleRow accepts (step % 16 == 0).
                        pt = pt_pool.tile([128, 2048], U8, tag="pt",
                                          name="pt")
                        pts.append(pt)
                        ptf8 = pt.bitcast(F8).rearrange(
                            "p (two c) -> p two c", c=1024)
                        for half in range(2):
                            i = 2 * p + half
                            stile = sps.tile([128, 1024], F32, tag="s",
                                             name="stile")
                            nc.tensor.matmul(
                                out=stile[:, 0:512],
                                lhsT=KT[hp][0:64, i * 128:(i + 1) * 128],
                                rhs=QT[hp][0:64, j * 512:(j + 1) * 512],
                                start=True, stop=True,
                                tile_position=(0, 0),
                            )
                            nc.tensor.matmul(
                                out=stile[:, 512:1024],
                                lhsT=KT[hp][64:128, i * 128:(i + 1) * 128],
                                rhs=QT[hp][64:128, j * 512:(j + 1) * 512],
                                start=True, stop=True,
                                tile_position=(64, 0),
                            )
                            emit_exp(eng,
                                     pt[:, half * 1024:(half + 1) * 1024],
                                     stile[:])
                            if half == 1:
                                pv_matmul(ctx_e, p, 2 * hp,
                                          ptf8[:, :, 0:512],
                                          start=(p == 0), stop=(p == NP - 1))
                            yield
                    # evict head-e ctx on this unit's own exp engine (its
                    # queue has no pending work => no priority inversion),
                    # freeing the bank for the next unit.
                    ctx_sb = rep_pool.tile([DH + 1, 1024], F32, tag="ctxsb",
                                           bufs=4)
                    if eng == "act":
                        nc.scalar.copy(out=ctx_sb[:, 0:512], in_=ctx_e[:])
                    else:
                        nc.vector.tensor_copy(out=ctx_sb[:, 0:512],
                                              in_=ctx_e[:])
                    yield
                    # head-o replay from the stored pt tiles (pure PE burst)
                    ctx_o = ctxps.tile([DH + 1, 512], F32, tag="ctx",
                                       name="ctx_o")
                    for p in range(NP):
                        ptf8 = pts[p].bitcast(F8).rearrange(
                            "p (two c) -> p two c", c=1024)
                        pv_matmul(ctx_o, p, 2 * hp + 1,
                                  ptf8[:, :, 512:1024],
                                  start=(p == 0), stop=(p == NP - 1))
                        if p % 4 == 3:
                            yield
                    if eng == "act":
                        nc.scalar.copy(out=ctx_sb[:, 512:1024], in_=ctx_o[:])
                    else:
                        nc.vector.tensor_copy(out=ctx_sb[:, 512:1024],
                                              in_=ctx_o[:])
                    # both denominator rows sit contiguous in ctx_sb row 64:
                    # ONE DMA round-trip feeds the partition-parallel
                    # reciprocal (bf16 out, feeds the PE broadcast).
                    l128 = rep_pool.tile([128, 8], F32, tag="l128", bufs=4)
                    nc.sync.dma_start(out=l128[:], in_=ctx_sb[DH:DH + 1, :])
                    l128b = rep_pool.tile([128, 8], BF16, tag="l128b", bufs=4)
                    with nc.allow_low_precision(
                            reason="bf16 recip row feeds the PE broadcast; "
                            "0.2% on 1/den is ~1e-4 at the logits"):
                        nc.vector.reciprocal(out=l128b[:], in_=l128[:])
                    rrow_b = rep_pool.tile([1, 1024], BF16, tag="rrow",
                                           bufs=4)
                    nc.sync.dma_start(out=rrow_b[:], in_=l128b[:])
                    pending_ops.append(
                        chain2_gen(7, eng, hp, j, ctx_sb, rrow_b))
                    yield

                def emit_outproj_m(j, m):
                    ps = sps.tile([128, 1024], F32, tag="s", name="ovps")
                    for kk in range(NE):
                        nc.tensor.matmul(
                            out=ps[:, 0:512],
                            lhsT=wo_sb[kk][:, m * 128:(m + 1) * 128],
                            rhs=CT[kk][:, j * 512:(j + 1) * 512],
                            start=(kk == 0),
                            stop=(kk == NE - 1),
                        )
                    if j == 0:
                        nc.vector.reduce_max(
                            out=pooled[m][:], in_=ps[:, 0:512], axis=X_AXIS,
                        )
                    else:
                        tmp = rep_pool.tile([128, 1], F32, tag="tmp")
                        nc.vector.reduce_max(
                            out=tmp[:], in_=ps[:, 0:512], axis=X_AXIS,
                        )
                        nc.vector.tensor_tensor(
                            out=pooled[m][:], in0=pooled[m][:],
                            in1=tmp[:], op=MAXOP,
                        )

                def outproj_gen(j, delay):
                    # Deferred so the next j's PE work is already queued in
                    # front of these CT-dependent matmuls — the normalize
                    # chains complete in the shadow of that work instead of
                    # stalling the in-order PE stream.
                    for _ in range(delay):
                        yield
                    for m in range(NE):
                        emit_outproj_m(j, m)
                        for _ in range(4):
                            yield

                # software-pipelined unit scheduler: 2 units in flight
                gens = deque()
                for j in range(NJ):
                    for hp in range(H // 2):
                        u = j * (H // 2) + hp
                        eng = ENG_UNITS[u % len(ENG_UNITS)]
                        gens.append((j, unit(j, hp, eng)))
                active = []
                pending_ops = []
                done_j = {j: 0 for j in range(NJ)}

                def pump_ops():
                    for g in list(pending_ops):
                        try:
                            next(g)
                        except StopIteration:
                            pending_ops.remove(g)

                while gens or active or pending_ops:
                    while len(active) < 2 and gens:
                        active.append(gens.popleft())
                    if not active:
                        pump_ops()
                        continue
                    j0, g = active.pop(0)
                    try:
                        next(g)
                        active.append((j0, g))
                    except StopIteration:
                        done_j[j0] += 1
                        if done_j[j0] == H // 2:
                            pending_ops.append(outproj_gen(j0, delay=20))
                    pump_ops()

                # classifier: bo is folded into bc on the host
                # (logits = max(ctx@Wo)@Wc + (bo@Wc + bc)).
                cls = sps.tile([128, 1024], F32, tag="s", name="clsps")
                for kk in range(NE):
                    nc.tensor.matmul(
                        out=cls[0:OUT, 0:1],
                        lhsT=wc_sb[kk][:],
                        rhs=pooled[kk][:],
                        start=(kk == 0),
                        stop=(kk == NE - 1),
                    )
                logits = fin_pool.tile([OUT, 1], F32, tag="logits")
                nc.vector.tensor_scalar_add(
                    out=logits[:], in0=cls[0:OUT, 0:1], scalar1=bc_sb[:]
                )
                nc.sync.dma_start(out=out[:, :], in_=logits[:])

    nc.finalize()
    return nc


def make_in_maps(inputs):
    """Shard the full inputs into per-core (per-batch-row) input dicts."""
    import ml_dtypes

    bf16 = ml_dtypes.bfloat16
    x = np.asarray(inputs["x"]).astype(np.int32)          # [B, S]
    S = x.shape[1]
    emb = np.ascontiguousarray(
        np.asarray(inputs["emb_table"], dtype=np.float32).astype(bf16))
    shared = {
        "emb": emb,
        "wq": np.ascontiguousarray(np.asarray(inputs["Wq"]).astype(bf16)),
        "wk": np.ascontiguousarray(np.asarray(inputs["Wk"]).astype(bf16)),
        "wv": np.ascontiguousarray(np.asarray(inputs["Wv"]).astype(bf16)),
        "wo": np.ascontiguousarray(np.asarray(inputs["Wo"]).astype(bf16)),
        "wc": np.ascontiguousarray(np.asarray(inputs["Wc"], dtype=np.float32)),
        "bq": np.ascontiguousarray(
            np.asarray(inputs["bq"], dtype=np.float32).reshape(4, 128).T),
        "bk": np.ascontiguousarray(
            np.asarray(inputs["bk"], dtype=np.float32).reshape(4, 128).T),
        "bo": np.ascontiguousarray(
            np.asarray(inputs["bo"], dtype=np.float32).reshape(4, 128).T),
        # bo folded into the classifier bias: logits = pooled@Wc + (bo@Wc+bc)
        "bc": np.ascontiguousarray(
            (np.asarray(inputs["bo"], dtype=np.float32)
             @ np.asarray(inputs["Wc"], dtype=np.float32)
             + np.asarray(inputs["bc"], dtype=np.float32)).reshape(OUT, 1)),
        "bv": np.ascontiguousarray(
            np.asarray(inputs["bv"]).astype(bf16).reshape(1, E)),
    }
    in_maps = []
    for c in range(x.shape[0]):
        xi = np.ascontiguousarray(x[c].reshape(S // 128, 128).T)  # [128, NT]
        in_maps.append({"xi": xi, **shared})
    return in_maps


_NC_CACHE = {}


def get_nc(S=2048, VOCAB=50257):
    key = (S, VOCAB)
    if key not in _NC_CACHE:
        _NC_CACHE[key] = build(S, VOCAB)
    return _NC_CACHE[key]


def run(inputs, trace=False):
    from concourse.bass_utils import run_bass_kernel_spmd

    nc = get_nc()
    in_maps = make_in_maps(inputs)
    res = run_bass_kernel_spmd(
        nc, in_maps, list(range(N_CORES)), trace=trace
    )
    outs = np.stack(
        [res.results[c]["out"].reshape(OUT) for c in range(N_CORES)]
    ).astype(np.float32)
    return outs, res


def kernel(**inputs):
    outs, _ = run(inputs, trace=False)
    return outs


# revision 69
# speedup vs baseline: 1.0739x; 1.0739x over previous
"""Trainium2 Bass kernel for MultiHeadSelfAttentionModelV1.

Model (per batch row):
    e   = emb_table[x]                      # [S, E]
    Q/K/V = e @ W* + b*                     # [S, E], split into H heads of Dh
    P_h = softmax(Q_h K_h^T / sqrt(Dh))
    ctx = concat_h(P_h V_h) @ Wo + bo       # [S, E]
    out = max_tokens(ctx) @ Wc + bc         # [OUT]

Sharding: pure data parallel over batch; B == n_cores == 8, one row per core.

Measured: ~370 us HW (vs 420 us baseline), rel err 8.1e-3 (gate 2e-2).
(A ~313 us variant using a stride-0 broadcast DMA from a DRAM bounce for
the softmax-denominator fanout was rejected: it contained the kernel's
only DMA-write -> DMA-read dependency edge and produced rare (~1/14
runs) corrupted outputs; all edges below are engine-mediated.)

Key design choices (numerics validated in numpy sim, sim == HW err):
 - emb table bf16 in DRAM; token gather via indirect DMA, PE-transpose in
   bf16, ONE strided eviction per token tile into a single eT supertile.
 - Q/K/V projections bf16 (fp8 DoubleRow was tried: only -4 us, err
   1.6e-2 — rejected). Bias folds into the PSUM eviction (ACT
   activation-Identity-bias for Q, DVE tensor_scalar_add for K).
 - V is evicted to fp8(e4m3) pair tiles va2[p][128 tok, 8 heads x 2
   k-subtiles x 80] laid out [V_h (64) | 1 | pad15] (DoubleRow ldweights
   needs subtile step % 16 == 0); the ones column makes the PV matmul
   accumulate the softmax denominator in PSUM row 64 for free.
 - softmax exp is ONE fused op per [128,1024] score tile:
   u8 = s*(1/ln2) + 55.35, bitcast u8 -> e4m3 IS exp(s/8) (Schraudolph in
   fp8 bit space). The quantization bias cancels in the softmax
   denominator because the denominator sums the same quantized P. Runs on
   ACT (activation Copy w/ scale+bias -> u8) or DVE (tensor_scalar
   mult,add -> u8), ONE engine per (j, head-pair) unit so every softmax
   row sees a single convert-rounding mode. GPSIMD cannot read PSUM on
   TRN2, so only these two engines can exp; ~145 us/engine is the floor.
 - PV runs fp8 DoubleRow: two 128-token k-tiles per matmul = 2x fewer PE
   streaming cycles. ctx accumulates [65, 512] fp32 in one PSUM bank.
 - Two-pass heads per unit: the chunk loop accumulates only head-e; the
   SBUF pt tiles are replayed through 8 more DoubleRow matmuls for head-o.
   This halves live ctx banks (2 instead of 4), buying a THIRD stile slot
   (sps bufs=3 x 2 banks + ctx 2 banks = all 8 PSUM banks) so scores(k+1)
   overlaps exp(k) instead of serializing on the slot.
 - Phase C is software-pipelined: two (j, head-pair) units in flight,
   ACT/DVE alternating, interleaved at chunk granularity. Q/K projection
   blocks are emitted lazily as unit prerequisites (their PE work fills
   exp-bound slack).
 - Normalization: ctx evicted on the unit's OWN exp engine (no priority
   inversion), both denominator rows land contiguous in ctx_sb row 64 ->
   one DMA to [128,8] for partition-parallel DVE reciprocal (bf16 out) ->
   one DMA back to a [1,1024] row -> PE outer-product ones^T @ row
   broadcasts it into a borrowed PSUM stile slot (the gpsimd
   partition_broadcast costs 5.5 us of ucode time and stalled the PE at
   every j boundary) -> own-engine copy to SBUF -> two GPSIMD multiplies
   write normalized bf16 ctx^T into CT. The outer-product + copy are
   emitted as a deferred pending-op (~7 scheduler steps) so the in-order
   PE stream never waits on the reciprocal row.
 - Output projection + maxpool run per j-chunk, emission deferred ~20
   scheduler steps so the normalize chains complete in the shadow of the
   next j's PE work. bo is folded into the classifier bias on the host
   (max commutes with the per-feature constant bo).
"""

import sys

import numpy as np

if "/opt/trn_rl_repo" not in sys.path:
    sys.path.insert(0, "/opt/trn_rl_repo")

from collections import deque

import concourse.bass as bass
import concourse.bacc as bacc
import concourse.tile as tile
from concourse import mybir
from concourse.masks import make_identity

F32 = mybir.dt.float32
BF16 = mybir.dt.bfloat16
F8 = mybir.dt.float8e4
U8 = mybir.dt.uint8
I32 = mybir.dt.int32
ADD = mybir.AluOpType.add
MULT = mybir.AluOpType.mult
MAXOP = mybir.AluOpType.max
IDENT_FN = mybir.ActivationFunctionType.Identity
COPY_FN = mybir.ActivationFunctionType.Copy
DR = mybir.MatmulPerfMode.DoubleRow
X_AXIS = mybir.AxisListType.X

# exp(s/8) ~= bitcast_e4m3(u8(s * SCH_A + SCH_B)); the e4m3 bit pattern of
# exp(s/8) is affine in s (Schraudolph). Tuned in sim; robust to the
# (unknown) HW round-vs-truncate convert mode since the resulting global
# scale on P cancels in the softmax denominator.
SCH_A = 1.4426950408889634
SCH_B = 55.35

B = 8
E = 512
H = 8
DH = 64
OUT = 10
N_CORES = 8

# Exp engine per (j, head-pair) unit. GPSIMD cannot read PSUM on TRN2, so
# only ACT and DVE can run the exp-convert. One engine per unit keeps each
# softmax row on a single convert-rounding mode; the window-2 scheduler
# keeps both engines busy on alternating units.
ENG_UNITS = ["act", "dve"]


def build(S=2048, VOCAB=50257):
    """Build the per-core Bass program (same program on all 8 cores)."""
    nc = bacc.Bacc()

    NT = S // 128   # 128-token tiles (16)
    NJ = S // 512   # 512-token q-chunks (4)
    NE = E // 128   # 128-feature chunks (4)
    NP = NT // 2    # pairs of token tiles for DoubleRow (8)

    xi = nc.declare_dram_parameter("xi", [128, NT], I32, isOutput=False)
    emb = nc.declare_dram_parameter("emb", [VOCAB, E], BF16, isOutput=False)
    wq = nc.declare_dram_parameter("wq", [E, E], BF16, isOutput=False)
    wk = nc.declare_dram_parameter("wk", [E, E], BF16, isOutput=False)
    wv = nc.declare_dram_parameter("wv", [E, E], BF16, isOutput=False)
    wo = nc.declare_dram_parameter("wo", [E, E], BF16, isOutput=False)
    wc = nc.declare_dram_parameter("wc", [E, OUT], F32, isOutput=False)
    bq = nc.declare_dram_parameter("bq", [128, NE], F32, isOutput=False)
    bk = nc.declare_dram_parameter("bk", [128, NE], F32, isOutput=False)
    bo = nc.declare_dram_parameter("bo", [128, NE], F32, isOutput=False)
    bv = nc.declare_dram_parameter("bv", [1, E], BF16, isOutput=False)
    bc = nc.declare_dram_parameter("bc", [OUT, 1], F32, isOutput=False)
    out = nc.declare_dram_parameter("out", [OUT, 1], F32, isOutput=True)

    with tile.TileContext(nc) as tc:
        with (
            tc.tile_pool(name="consts", bufs=1) as consts,
            tc.tile_pool(name="qkT", bufs=1) as qkT_pool,
            tc.tile_pool(name="va2p", bufs=1) as va2_pool,
            tc.tile_pool(name="ctxT", bufs=1) as ctxT_pool,
            tc.tile_pool(name="eTp", bufs=1) as eT_pool,
            tc.tile_pool(name="projw", bufs=1) as projw,
            tc.tile_pool(name="fin", bufs=1) as fin_pool,
        ):
            # ---- constants (emission order = DMA priority: index + QKV
            # weights first so the gather/projection pipeline starts ASAP)
            idx_sb = consts.tile([128, NT], I32, tag="idx")
            nc.sync.dma_start(out=idx_sb, in_=xi[:, :])
            wq_sb = [projw.tile([128, E], BF16, tag=f"wq{k}", name=f"wq{k}")
                     for k in range(NE)]
            wk_sb = [projw.tile([128, E], BF16, tag=f"wk{k}", name=f"wk{k}")
                     for k in range(NE)]
            wv_sb = [projw.tile([128, E], BF16, tag=f"wv{k}", name=f"wv{k}")
                     for k in range(NE)]
            for k in range(NE):
                nc.sync.dma_start(out=wv_sb[k], in_=wv[k * 128:(k + 1) * 128, :])
                nc.sync.dma_start(out=wk_sb[k], in_=wk[k * 128:(k + 1) * 128, :])
                nc.sync.dma_start(out=wq_sb[k], in_=wq[k * 128:(k + 1) * 128, :])
            ident = consts.tile([128, 128], BF16, tag="ident")
            make_identity(nc, ident)
            wo_sb = [consts.tile([128, E], BF16, tag=f"wo{k}", name=f"wo{k}")
                     for k in range(NE)]
            for k in range(NE):
                nc.sync.dma_start(out=wo_sb[k], in_=wo[k * 128:(k + 1) * 128, :])
            wc_sb = [consts.tile([128, OUT], F32, tag=f"wc{k}", name=f"wc{k}")
                     for k in range(NE)]
            for k in range(NE):
                nc.sync.dma_start(out=wc_sb[k], in_=wc[k * 128:(k + 1) * 128, :])
            bq_sb = consts.tile([128, NE], F32, tag="bq")
            nc.sync.dma_start(out=bq_sb, in_=bq[:, :])
            bk_sb = consts.tile([128, NE], F32, tag="bk")
            nc.sync.dma_start(out=bk_sb, in_=bk[:, :])
            bo_sb = consts.tile([128, NE], F32, tag="bo")
            nc.sync.dma_start(out=bo_sb, in_=bo[:, :])
            bv_sb = consts.tile([1, E], BF16, tag="bv")
            nc.sync.dma_start(out=bv_sb, in_=bv[:, :])
            bc_sb = consts.tile([OUT, 1], F32, tag="bc")
            nc.sync.dma_start(out=bc_sb, in_=bc[:, :])
            ones_row = consts.tile([1, 128], BF16, tag="ones")
            nc.vector.memset(ones_row, 1.0)
            # e^T as ONE tile [128, NE*S] bf16 (feature chunk kk at columns
            # kk*S..): lets each token tile evict with a single strided copy.
            eT = eT_pool.tile([128, NE * S], BF16, tag="eT", name="eT")

            # persistent activations
            QT = [qkT_pool.tile([128, S], BF16, tag=f"qt{k}", name=f"qt{k}")
                  for k in range(NE)]
            KT = [qkT_pool.tile([128, S], BF16, tag=f"kt{k}", name=f"kt{k}")
                  for k in range(NE)]
            # V fp8 pair tiles: [128 tok, H * (2 k-subtiles * 80)]; per head
            # two [V_h | 1 | pad] blocks at stride 80 (DoubleRow ldweights
            # requires subtile step % 16 == 0). Preset to 1.0 so the ones
            # columns stay; pad columns are never read.
            va2 = [va2_pool.tile([128, H * 160], F8, tag=f"va{p}",
                                 name=f"va{p}") for p in range(NP)]
            CT = [ctxT_pool.tile([128, S], BF16, tag=f"ct{k}", name=f"ct{k}")
                  for k in range(NE)]

            # ============ phase A: gather, eT, V projection ============
            with (
                tc.tile_pool(name="enat", bufs=3) as enat_pool,
                tc.tile_pool(name="tps", bufs=2, space="PSUM") as tps,
                tc.tile_pool(name="qkvps", bufs=4, space="PSUM") as qkvps,
            ):
                for p in range(NP):
                    nc.vector.memset(va2[p][:], 1.0)

                evict_rr = ["dve", "act"]
                for t in range(NT):
                    e_nat = enat_pool.tile([128, E], BF16)
                    nc.gpsimd.indirect_dma_start(
                        out=e_nat[:],
                        out_offset=None,
                        in_=emb[:, :],
                        in_offset=bass.IndirectOffsetOnAxis(
                            ap=idx_sb[:, t:t + 1], axis=0
                        ),
                    )
                    # 4 transposes collect in one PSUM tile -> ONE eviction
                    tp = tps.tile([128, 512], BF16)
                    for f in range(NE):
                        nc.tensor.transpose(
                            out=tp[:, f * 128:(f + 1) * 128],
                            in_=e_nat[:, f * 128:(f + 1) * 128],
                            identity=ident[:],
                        )
                    dst = eT[:].rearrange(
                        "p (f s) -> p f s", s=S)[:, :, t * 128:(t + 1) * 128]
                    src = tp[:].rearrange("p (f c) -> p f c", c=128)
                    if evict_rr[t % 2] == "dve":
                        nc.vector.tensor_copy(out=dst, in_=src)
                    else:
                        nc.scalar.copy(out=dst, in_=src)

                # V token-major -> fp8 pair tiles (+bv via ones-row matmul)
                for t in range(NT):
                    ps = qkvps.tile([128, 512], F32, tag="qkv")
                    for kk in range(NE):
                        nc.tensor.matmul(
                            out=ps[:],
                            lhsT=eT[:, kk * S + t * 128:kk * S + (t + 1) * 128],
                            rhs=wv_sb[kk][:],
                            start=(kk == 0),
                            stop=False,
                        )
                    nc.tensor.matmul(
                        out=ps[:], lhsT=ones_row[:], rhs=bv_sb[:],
                        start=False, stop=True,
                    )
                    p, half = divmod(t, 2)
                    dst = va2[p][:].rearrange(
                        "p (h two c) -> p h two c", two=2, c=80)
                    nc.scalar.copy(
                        out=dst[:, :, half, 0:DH],
                        in_=ps[:].rearrange("p (h c) -> p h c", c=DH),
                    )

            # ============ phase C+D: attention, out-proj, pool ============
            with (
                tc.tile_pool(name="ptp", bufs=18) as pt_pool,
                tc.tile_pool(name="rep", bufs=2) as rep_pool,
                tc.tile_pool(name="sps", bufs=3, space="PSUM") as sps,
                tc.tile_pool(name="ctxps", bufs=2, space="PSUM") as ctxps,
            ):
                pooled = [fin_pool.tile([128, 1], F32, tag=f"pool{m}",
                                        name=f"pool{m}") for m in range(NE)]

                def emit_exp(eng, dst_u8, src):
                    if eng == "act":
                        nc.scalar.activation(
                            out=dst_u8, in_=src, func=COPY_FN,
                            scale=SCH_A, bias=SCH_B,
                        )
                    elif eng == "dve":
                        nc.vector.tensor_scalar(
                            out=dst_u8, in0=src,
                            scalar1=SCH_A, scalar2=SCH_B, op0=MULT, op1=ADD,
                        )
                    else:
                        nc.gpsimd.tensor_scalar(
                            out=dst_u8, in0=src,
                            scalar1=SCH_A, scalar2=SCH_B, op0=MULT, op1=ADD,
                        )

                # Q/K projection blocks are emitted lazily inside phase C,
                # right before the first unit that needs them — their PE
                # work fills exp-bound pipeline slack. They borrow sps slots.
                emitted_k = set()
                emitted_q = set()

                def emit_k_block(m, jj):
                    ps = sps.tile([128, 1024], F32, tag="s", name="kps")
                    for kk in range(NE):
                        nc.tensor.matmul(
                            out=ps[:, 0:512],
                            lhsT=wk_sb[kk][:, m * 128:(m + 1) * 128],
                            rhs=eT[:, kk * S + jj * 512:kk * S + (jj + 1) * 512],
                            start=(kk == 0),
                            stop=(kk == NE - 1),
                        )
                    nc.vector.tensor_scalar_add(
                        out=KT[m][:, jj * 512:(jj + 1) * 512],
                        in0=ps[:, 0:512], scalar1=bk_sb[:, m:m + 1],
                    )

                def emit_q_block(m, jj):
                    ps = sps.tile([128, 1024], F32, tag="s", name="qps")
                    for kk in range(NE):
                        nc.tensor.matmul(
                            out=ps[:, 0:512],
                            lhsT=wq_sb[kk][:, m * 128:(m + 1) * 128],
                            rhs=eT[:, kk * S + jj * 512:kk * S + (jj + 1) * 512],
                            start=(kk == 0),
                            stop=(kk == NE - 1),
                        )
                    nc.scalar.activation(
                        out=QT[m][:, jj * 512:(jj + 1) * 512],
                        in_=ps[:, 0:512], func=IDENT_FN,
                        bias=bq_sb[:, m:m + 1], scale=1.0,
                    )

                def ensure_proj(j, hp):
                    if hp not in emitted_k:
                        emitted_k.add(hp)
                        for jj in range(NJ):
                            emit_k_block(hp, jj)
                            yield
                    if (j, hp) not in emitted_q:
                        emitted_q.add((j, hp))
                        emit_q_block(hp, j)
                        yield

                def chain2_gen(delay, eng, hp, j, ctx_sb, rrow_b):
                    # Deferred second half of the normalize chain: by the
                    # time the PE reaches the outer-product, the reciprocal
                    # row is ready (no in-order PE stall). Every edge here
                    # is engine-mediated — no DMA-write -> DMA-read pairs.
                    for _ in range(delay):
                        yield
                    # broadcast = PE outer product ones[1,64]^T @ rrow[1,.]
                    rep_ps = sps.tile([128, 1024], F32, tag="s", name="repps")
                    for half in range(2):
                        nc.tensor.matmul(
                            out=rep_ps[0:64, half * 512:(half + 1) * 512],
                            lhsT=ones_row[0:1, 0:64],
                            rhs=rrow_b[:, half * 512:(half + 1) * 512],
                            start=True, stop=True,
                        )
                    rep_sb = rep_pool.tile([64, 1024], F32, tag="rep",
                                           bufs=4)
                    if eng == "act":
                        nc.scalar.copy(out=rep_sb[:], in_=rep_ps[0:64, :])
                    else:
                        nc.vector.tensor_copy(out=rep_sb[:],
                                              in_=rep_ps[0:64, :])
                    for off, h in ((0, 2 * hp), (512, 2 * hp + 1)):
                        nc.gpsimd.tensor_tensor(
                            out=CT[hp][(h % 2) * 64:(h % 2) * 64 + 64,
                                       j * 512:(j + 1) * 512],
                            in0=ctx_sb[0:DH, off:off + 512],
                            in1=rep_sb[:, off:off + 512],
                            op=MULT,
                        )

                def pv_matmul(ctx, p, h, ptf8, start, stop):
                    nc.tensor.matmul(
                        out=ctx[:],
                        lhsT=va2[p][:, h * 160:(h + 1) * 160]
                        .rearrange("p (two c) -> p two c", c=80)[:, :, 0:65],
                        rhs=ptf8,
                        start=start, stop=stop,
                        perf_mode=DR,
                        skip_group_check=True,
                    )

                def unit(j, hp, eng):
                    """One (j, head-pair) attention unit; yields per chunk.

                    Two-pass over heads: the chunk loop accumulates only
                    head-e (1 PSUM bank); the stored SBUF pt tiles are then
                    replayed through 8 more DR matmuls for head-o. This
                    halves live ctx banks, buying a third stile slot so
                    scores(k+1) overlaps exp(k).
                    """
                    yield from ensure_proj(j, hp)
                    ctx_e = ctxps.tile([DH + 1, 512], F32, tag="ctx",
                                       name="ctx_e")
                    pts = []
                    for p in range(NP):
                        # pt layout [128, (k-subtile, head, q)] = the raw
                        # concatenation of the pair's two exp outputs; each
                        # head's two k-subtile P blocks sit at stride 1024,
                        # which DoubleRow accepts (step % 16 == 0).
                        pt = pt_pool.tile([128, 2048], U8, tag="pt",
                                          name="pt")
                        pts.append(pt)
                        ptf8 = pt.bitcast(F8).rearrange(
                            "p (two c) -> p two c", c=1024)
                        for half in range(2):
                            i = 2 * p + half
                            stile = sps.tile([128, 1024], F32, tag="s",
                                             name="stile")
                            nc.tensor.matmul(
                                out=stile[:, 0:512],
                                lhsT=KT[hp][0:64, i * 128:(i + 1) * 128],
                                rhs=QT[hp][0:64, j * 512:(j + 1) * 512],
                                start=True, stop=True,
                                tile_position=(0, 0),
                            )
                            nc.tensor.matmul(
                                out=stile[:, 512:1024],
                                lhsT=KT[hp][64:128, i * 128:(i + 1) * 128],
                                rhs=QT[hp][64:128, j * 512:(j + 1) * 512],
                                start=True, stop=True,
                                tile_position=(64, 0),
                            )
                            emit_exp(eng,
                                     pt[:, half * 1024:(half + 1) * 1024],
                                     stile[:])
                            if half == 1:
                                pv_matmul(ctx_e, p, 2 * hp,
                                          ptf8[:, :, 0:512],
                                          start=(p == 0), stop=(p == NP - 1))
                            yield
                    # evict head-e ctx on this unit's own exp engine (its
                    # queue has no pending work => no priority inversion),
                    # freeing the bank for the next unit.
                    ctx_sb = rep_pool.tile([DH + 1, 1024], F32, tag="ctxsb",
                                           bufs=4)
                    if eng == "act":
                        nc.scalar.copy(out=ctx_sb[:, 0:512], in_=ctx_e[:])
                    else:
                        nc.vector.tensor_copy(out=ctx_sb[:, 0:512],
                                              in_=ctx_e[:])
                    yield
                    # head-o replay from the stored pt tiles (pure PE burst)
                    ctx_o = ctxps.tile([DH + 1, 512], F32, tag="ctx",
                                       name="ctx_o")
                    for p in range(NP):
                        ptf8 = pts[p].bitcast(F8).rearrange(
                            "p (two c) -> p two c", c=1024)
                        pv_matmul(ctx_o, p, 2 * hp + 1,
                                  ptf8[:, :, 512:1024],
                                  start=(p == 0), stop=(p == NP - 1))
                        if p % 4 == 3:
                            yield
                    if eng == "act":
                        nc.scalar.copy(out=ctx_sb[:, 512:1024], in_=ctx_o[:])
                    else:
                        nc.vector.tensor_copy(out=ctx_sb[:, 512:1024],
                                              in_=ctx_o[:])
                    # both denominator rows sit contiguous in ctx_sb row 64:
                    # ONE DMA round-trip feeds the partition-parallel
                    # reciprocal (bf16 out, feeds the PE broadcast).
                    l128 = rep_pool.tile([128, 8], F32, tag="l128", bufs=4)
                    nc.sync.dma_start(out=l128[:], in_=ctx_sb[DH:DH + 1, :])
                    l128b = rep_pool.tile([128, 8], BF16, tag="l128b", bufs=4)
                    with nc.allow_low_precision(
                            reason="bf16 recip row feeds the PE broadcast; "
                            "0.2% on 1/den is ~1e-4 at the logits"):
                        nc.vector.reciprocal(out=l128b[:], in_=l128[:])
                    rrow_b = rep_pool.tile([1, 1024], BF16, tag="rrow",
                                           bufs=4)
                    nc.sync.dma_start(out=rrow_b[:], in_=l128b[:])
                    pending_ops.append(
                        chain2_gen(12, eng, hp, j, ctx_sb, rrow_b))
                    yield

                def emit_outproj_m(j, m):
                    ps = sps.tile([128, 1024], F32, tag="s", name="ovps")
                    for kk in range(NE):
                        nc.tensor.matmul(
                            out=ps[:, 0:512],
                            lhsT=wo_sb[kk][:, m * 128:(m + 1) * 128],
                            rhs=CT[kk][:, j * 512:(j + 1) * 512],
                            start=(kk == 0),
                            stop=(kk == NE - 1),
                        )
                    if j == 0:
                        nc.vector.reduce_max(
                            out=pooled[m][:], in_=ps[:, 0:512], axis=X_AXIS,
                        )
                    else:
                        tmp = rep_pool.tile([128, 1], F32, tag="tmp")
                        nc.vector.reduce_max(
                            out=tmp[:], in_=ps[:, 0:512], axis=X_AXIS,
                        )
                        nc.vector.tensor_tensor(
                            out=pooled[m][:], in0=pooled[m][:],
                            in1=tmp[:], op=MAXOP,
                        )

                def outproj_gen(j, delay):
                    # Deferred so the next j's PE work is already queued in
                    # front of these CT-dependent matmuls — the normalize
                    # chains complete in the shadow of that work instead of
                    # stalling the in-order PE stream.
                    for _ in range(delay):
                        yield
                    for m in range(NE):
                        emit_outproj_m(j, m)
                        for _ in range(4):
                            yield

                # software-pipelined unit scheduler: 2 units in flight
                gens = deque()
                for j in range(NJ):
                    for hp in range(H // 2):
                        u = j * (H // 2) + hp
                        eng = ENG_UNITS[u % len(ENG_UNITS)]
                        gens.append((j, unit(j, hp, eng)))
                active = []
                pending_ops = []
                done_j = {j: 0 for j in range(NJ)}

                def pump_ops():
                    for g in list(pending_ops):
                        try:
                            next(g)
                        except StopIteration:
                            pending_ops.remove(g)

                while gens or active or pending_ops:
                    while len(active) < 2 and gens:
                        active.append(gens.popleft())
                    if not active:
                        pump_ops()
                        continue
                    j0, g = active.pop(0)
                    try:
                        next(g)
                        active.append((j0, g))
                    except StopIteration:
                        done_j[j0] += 1
                        if done_j[j0] == H // 2:
                            pending_ops.append(outproj_gen(j0, delay=24))
                    pump_ops()

                # classifier: bo is folded into bc on the host
                # (logits = max(ctx@Wo)@Wc + (bo@Wc + bc)).
                cls = sps.tile([128, 1024], F32, tag="s", name="clsps")
                for kk in range(NE):
                    nc.tensor.matmul(
                        out=cls[0:OUT, 0:1],
                        lhsT=wc_sb[kk][:],
                        rhs=pooled[kk][:],
                        start=(kk == 0),
                        stop=(kk == NE - 1),
                    )
                logits = fin_pool.tile([OUT, 1], F32, tag="logits")
                nc.vector.tensor_scalar_add(
                    out=logits[:], in0=cls[0:OUT, 0:1], scalar1=bc_sb[:]
                )
                nc.sync.dma_start(out=out[:, :], in_=logits[:])

    nc.finalize()
    return nc


def make_in_maps(inputs):
    """Shard the full inputs into per-core (per-batch-row) input dicts."""
    import ml_dtypes

    bf16 = ml_dtypes.bfloat16
    x = np.asarray(inputs["x"]).astype(np.int32)          # [B, S]
    S = x.shape[1]
    emb = np.ascontiguousarray(
        np.asarray(inputs["emb_table"], dtype=np.float32).astype(bf16))
    shared = {
        "emb": emb,
        "wq": np.ascontiguousarray(np.asarray(inputs["Wq"]).astype(bf16)),
        "wk": np.ascontiguousarray(np.asarray(inputs["Wk"]).astype(bf16)),
        "wv": np.ascontiguousarray(np.asarray(inputs["Wv"]).astype(bf16)),
        "wo": np.ascontiguousarray(np.asarray(inputs["Wo"]).astype(bf16)),
        "wc": np.ascontiguousarray(np.asarray(inputs["Wc"], dtype=np.float32)),
        "bq": np.ascontiguousarray(
            np.asarray(inputs["bq"], dtype=np.float32).reshape(4, 128).T),
        "bk": np.ascontiguousarray(
            np.asarray(inputs["bk"], dtype=np.float32).reshape(4, 128).T),
        "bo": np.ascontiguousarray(
            np.asarray(inputs["bo"], dtype=np.float32).reshape(4, 128).T),
        # bo folded into the classifier bias: logits = pooled@Wc + (bo@Wc+bc)
        "bc": np.ascontiguousarray(
            (np.asarray(inputs["bo"], dtype=np.float32)
             @ np.asarray(inputs["Wc"], dtype=np.float32)
             + np.asarray(inputs["bc"], dtype=np.float32)).reshape(OUT, 1)),
        "bv": np.ascontiguousarray(
            np.asarray(inputs["bv"]).astype(bf16).reshape(1, E)),
    }
    in_maps = []
    for c in range(x.shape[0]):
        xi = np.ascontiguousarray(x[c].reshape(S // 128, 128).T)  # [128, NT]
        in_maps.append({"xi": xi, **shared})
    return in_maps


_NC_CACHE = {}


def get_nc(S=2048, VOCAB=50257):
    key = (S, VOCAB)
    if key not in _NC_CACHE:
        _NC_CACHE[key] = build(S, VOCAB)
    return _NC_CACHE[key]


def run(inputs, trace=False):
    from concourse.bass_utils import run_bass_kernel_spmd

    nc = get_nc()
    in_maps = make_in_maps(inputs)
    res = run_bass_kernel_spmd(
        nc, in_maps, list(range(N_CORES)), trace=trace
    )
    outs = np.stack(
        [res.results[c]["out"].reshape(OUT) for c in range(N_CORES)]
    ).astype(np.float32)
    return outs, res


def kernel(**inputs):
    outs, _ = run(inputs, trace=False)
    return outs
